# revision 1
# baseline (speedup 1.0000x reference)
"""Trainium2 Bass kernel for MixLoRA sparse MoE (8 experts, top-2, shared base MLP).

Sharding: 2D — 4-way over tokens (512 each) x 2-way over the hidden dim H
(2048 each). Every core computes its token-quarter's router + fc1/expert
work over its H-half, plus a PARTIAL fc2 (W2 and B2 contractions over its
H-half); the host sums the H-pair partials (b2 is added by the hh==0 core
only). This makes every matmul N=512 (amortizes LDWEIGHTS + ACT overhead).

Per-core pipeline (feature-major: partitions = feature slice, free = tokens):
  - Router in fp32: logits -> top2 -> w1 = sigmoid(l1-l2); per-expert dense
    weights replicated across partitions via selector matmuls.
  - common fc1 in PSUM once per (H-slice); per-expert LoRA deltas chained
    in place via difference matmuls  F_e = F_{e-1} + (2B1[e]^T u_e - 2B1[e-1]^T u_{e-1}).
  - a_e = silu(F_e + b1) on ScalarE (bias folds b1, reads PSUM directly).
  - ca_e = c_e * a_e on DVE; abar += ca_e on GpSimd; z_e = A2[e] @ ca_e via
    column-tiled packed matmuls (4 experts concurrent per PSUM bank).
  - out_partial = W2half^T @ abar + sum_s B2stack_s^T z_s (+ b2 on hh==0).
All big matmuls bf16 (fp32 accumulate); router fp32.
"""

import sys, os
sys.path.insert(0, "/opt/trn_rl_repo")

from contextlib import ExitStack

import numpy as np
import ml_dtypes

import concourse.bass as bass
import concourse.tile as tile
from concourse import mybir, bacc
from concourse.bass_utils import run_bass_kernel_spmd
from concourse.masks import make_identity

BF = ml_dtypes.bfloat16

NCORES = 8
TQ = 4               # token shards
HH = 2               # H shards
D, H, E, R = 1024, 4096, 8, 16
NT = 2048
T = NT // TQ         # tokens per core (512)
HL = H // HH         # H per core (2048)
KD = D // 128        # 8
MH = HL // 128       # 16 local H slices
MD = D // 128        # 8
SC = 2.0
MCHUNK = 2
NCH = MH // MCHUNK   # 8

f32 = mybir.dt.float32
bf16 = mybir.dt.bfloat16


def _build_bass(slots=8):
    nc = bacc.Bacc("TRN2", target_bir_lowering=False, debug=False)

    xtf = nc.dram_tensor("xtf", [128, KD * T], f32, kind="ExternalInput")
    xtb = nc.dram_tensor("xtb", [128, KD * T], bf16, kind="ExternalInput")
    gt = nc.dram_tensor("gt", [128, KD * E], f32, kind="ExternalInput")
    w1p = nc.dram_tensor("w1p", [MH, 128, KD * 128], bf16, kind="ExternalInput")
    w2p = nc.dram_tensor("w2p", [MD, 128, MH * 128], bf16, kind="ExternalInput")
    a1s = nc.dram_tensor("a1s", [128, KD * 256], bf16, kind="ExternalInput")
    b1d = nc.dram_tensor("b1d", [2, 128, HL], bf16, kind="ExternalInput")
    a2s = nc.dram_tensor("a2s", [128, MH * 256], bf16, kind="ExternalInput")
    b2s = nc.dram_tensor("b2s", [2, 128, D], bf16, kind="ExternalInput")
    b1c = nc.dram_tensor("b1c", [128, MH], f32, kind="ExternalInput")
    b2c = nc.dram_tensor("b2c", [128, MD], f32, kind="ExternalInput")
    sel = nc.dram_tensor("sel", [8, 8 * 128], bf16, kind="ExternalInput")
    outt = nc.dram_tensor("outt", [128, MD * T], f32, kind="ExternalOutput")

    with tile.TileContext(nc) as tc, ExitStack() as ctx:
        consts = ctx.enter_context(tc.tile_pool(name="consts", bufs=1))
        wpool = ctx.enter_context(tc.tile_pool(name="wpool", bufs=4))
        w2pool = ctx.enter_context(tc.tile_pool(name="w2pool", bufs=3))
        abufs = ctx.enter_context(tc.tile_pool(name="abufs", bufs=12))
        cabufs = ctx.enter_context(tc.tile_pool(name="cabufs", bufs=8))
        small = ctx.enter_context(tc.tile_pool(name="small", bufs=2))
        outp = ctx.enter_context(tc.tile_pool(name="outp", bufs=3))
        psMM = ctx.enter_context(tc.tile_pool(name="psMM", bufs=5, space="PSUM"))
        psZ = ctx.enter_context(tc.tile_pool(name="psZ", bufs=1, space="PSUM"))
        psM = ctx.enter_context(tc.tile_pool(name="psM", bufs=1, space="PSUM"))

        xtf_sb = consts.tile([128, KD * T], f32, tag="xtf_sb")
        xtb_sb = consts.tile([128, KD * T], bf16, tag="xtb_sb")
        for k in range(KD):
            nc.sync.dma_start(xtf_sb[:, k * T:(k + 1) * T], xtf[:, k * T:(k + 1) * T])
            nc.sync.dma_start(xtb_sb[:, k * T:(k + 1) * T], xtb[:, k * T:(k + 1) * T])
        gt_sb = consts.tile([128, KD * E], f32, tag="gt_sb")
        nc.sync.dma_start(gt_sb, gt[:])
        a1s_sb = consts.tile([128, KD * 256], bf16, tag="a1s_sb")
        nc.sync.dma_start(a1s_sb, a1s[:])
        b1d_sb = [consts.tile([128, HL], bf16, tag=f"b1d{s}", name=f"b1d_sb{s}")
                  for s in range(2)]
        for s in range(2):
            nc.sync.dma_start(b1d_sb[s], b1d[s])
        a2s_sb = consts.tile([128, MH * 256], bf16, tag="a2s_sb")
        nc.sync.dma_start(a2s_sb, a2s[:])
        b2s_sb = [consts.tile([128, D], bf16, tag=f"b2s{s}", name=f"b2s_sb{s}")
                  for s in range(2)]
        for s in range(2):
            nc.sync.dma_start(b2s_sb[s], b2s[s])
        b1c_sb = consts.tile([128, MH], f32, tag="b1c_sb")
        nc.sync.dma_start(b1c_sb, b1c[:])
        b2c_sb = consts.tile([128, MD], f32, tag="b2c_sb")
        nc.sync.dma_start(b2c_sb, b2c[:])
        sel_sb = consts.tile([8, E * 128], bf16, tag="sel_sb")
        nc.sync.dma_start(sel_sb, sel[:])
        ident = consts.tile([128, 128], f32, tag="ident")
        make_identity(nc, ident)
        identb = consts.tile([128, 128], bf16, tag="identb")
        make_identity(nc, identb)

        def xtf_k(k, tt):
            return xtf_sb[:, k * T + tt * 128:k * T + (tt + 1) * 128]

        def xtb_k(k):
            return xtb_sb[:, k * T:(k + 1) * T]

        # ---- chunk fc1 fills (function so chunk 0 can precede the router) ----
        fps_by_ch = {}

        def emit_fills(ch):
            m0 = ch * MCHUNK
            fps = {}
            for mi in range(MCHUNK):
                m = m0 + mi
                w1m = wpool.tile([128, KD * 128], bf16, tag="w1m", name="w1m")
                nc.sync.dma_start(w1m, w1p[m])
                f_ps = psMM.tile([128, T], f32, tag="mm", name="f_ps")
                fps[mi] = f_ps
                for k in range(KD):
                    nc.tensor.matmul(f_ps, w1m[:, k * 128:(k + 1) * 128], xtb_k(k),
                                     start=(k == 0), stop=False)
            fps_by_ch[ch] = fps

        # ---- Router (fp32): logits matmuls, then batched top-2 math ----
        NTT = T // 128
        lgall = small.tile([128, NTT * 8], f32, tag="lgall")
        for tt in range(NTT):
            lg_ps = psM.tile([128, 8], f32, tag="misc", name="lg_ps")
            for k in range(KD):
                nc.tensor.matmul(lg_ps, xtf_k(k, tt), gt_sb[:, k * E:(k + 1) * E],
                                 start=(k == 0), stop=(k == KD - 1))
            nc.vector.tensor_copy(lgall[:, tt * 8:(tt + 1) * 8], lg_ps)

        emit_fills(0)
        emit_fills(1)

        def bc4(v):            # [128, NTT] -> [128, NTT, 8] broadcast AP
            return bass.AP(tensor=v.tensor, offset=v.offset,
                           ap=[list(v.ap[0]), [1, NTT], [0, 8]])

        lg3 = lgall.rearrange("p (t e) -> p t e", t=NTT)
        m1 = small.tile([128, NTT], f32, tag="m1")
        nc.vector.tensor_reduce(m1, lg3, axis=mybir.AxisListType.X,
                                op=mybir.AluOpType.max)
        mask1 = small.tile([128, NTT * 8], f32, tag="mask1")
        nc.vector.tensor_tensor(mask1.rearrange("p (t e) -> p t e", t=NTT),
                                lg3, bc4(m1), op=mybir.AluOpType.is_equal)
        tmp = small.tile([128, NTT * 8], f32, tag="tmp8")
        nc.vector.scalar_tensor_tensor(tmp, mask1, -1e30, lgall,
                                       op0=mybir.AluOpType.mult,
                                       op1=mybir.AluOpType.add)
        m2 = small.tile([128, NTT], f32, tag="m2")
        nc.vector.tensor_reduce(m2, tmp.rearrange("p (t e) -> p t e", t=NTT),
                                axis=mybir.AxisListType.X, op=mybir.AluOpType.max)
        mask2 = small.tile([128, NTT * 8], f32, tag="mask2")
        nc.vector.tensor_tensor(mask2.rearrange("p (t e) -> p t e", t=NTT),
                                tmp.rearrange("p (t e) -> p t e", t=NTT),
                                bc4(m2), op=mybir.AluOpType.is_equal)
        dm = small.tile([128, NTT], f32, tag="dm")
        nc.vector.tensor_tensor(dm, m1, m2, op=mybir.AluOpType.subtract)
        wa = small.tile([128, NTT], f32, tag="wa")
        nc.scalar.activation(wa, dm, mybir.ActivationFunctionType.Sigmoid)
        wb = small.tile([128, NTT], f32, tag="wb")
        nc.vector.tensor_scalar(wb, wa, -1.0, 1.0,
                                op0=mybir.AluOpType.mult,
                                op1=mybir.AluOpType.add)
        c1 = small.tile([128, NTT * 8], f32, tag="c1")
        nc.vector.tensor_tensor(c1.rearrange("p (t e) -> p t e", t=NTT),
                                mask1.rearrange("p (t e) -> p t e", t=NTT),
                                bc4(wa), op=mybir.AluOpType.mult)
        c2 = small.tile([128, NTT * 8], f32, tag="c2")
        nc.vector.tensor_tensor(c2.rearrange("p (t e) -> p t e", t=NTT),
                                mask2.rearrange("p (t e) -> p t e", t=NTT),
                                bc4(wb), op=mybir.AluOpType.mult)
        cmatall = small.tile([128, NTT * 8], f32, tag="cmatall")
        nc.vector.tensor_tensor(cmatall, c1, c2, op=mybir.AluOpType.add)

        cT = small.tile([8, T], f32, tag="cT")
        for tt in range(NTT):
            cT_ps = psM.tile([8, 128], f32, tag="misc", name="cT_ps")
            nc.tensor.transpose(cT_ps, cmatall[:, tt * 8:(tt + 1) * 8], ident)
            nc.vector.tensor_copy(cT[:, tt * 128:(tt + 1) * 128], cT_ps)

        cTbf = small.tile([8, T], bf16, tag="cTbf")
        nc.vector.tensor_copy(cTbf, cT)
        cbc = consts.tile([128, slots * T], bf16, tag="cbc")
        for e in range(slots):
            cb_ps = psM.tile([128, T], f32, tag="misc", name="ms_ps")
            nc.tensor.matmul(cb_ps, sel_sb[:, e * 128:(e + 1) * 128], cTbf,
                             start=True, stop=True)
            nc.vector.tensor_copy(cbc[:, e * T:(e + 1) * T], cb_ps)

        # ---- u pairs ----
        up_sb = []
        for s in range(2):
            u_ps = psM.tile([128, T], f32, tag="misc", name="u_ps")
            for k in range(KD):
                nc.tensor.matmul(u_ps, a1s_sb[:, k * 256 + s * 128:k * 256 + (s + 1) * 128],
                                 xtb_k(k), start=(k == 0), stop=(k == KD - 1))
            u_sb = consts.tile([128, T], bf16, tag=f"u{s}", name=f"u_sb{s}")
            nc.vector.tensor_copy(u_sb, u_ps)
            up_sb.append(u_sb)

        # ---- fc1 + expert chain + weighting ----
        abar = consts.tile([128, MH * T], bf16, tag="abar")
        zps = [psZ.tile([128, T], f32, tag=f"z{s}", name=f"zps{s}") for s in range(2)]
        for ch in range(NCH):
            m0 = ch * MCHUNK
            asl = {}
            if ch not in fps_by_ch:
                emit_fills(ch)
            fps = fps_by_ch.pop(ch)
            for e in range(slots):
                asl[e] = abufs.tile([128, MCHUNK * T], bf16, tag="a", name=f"asl{e}")
                s, g = divmod(e, 4)
                for mi in range(MCHUNK):
                    m = m0 + mi
                    nc.tensor.matmul(
                        fps[mi],
                        b1d_sb[s][32 * g:32 * g + 32, m * 128:(m + 1) * 128],
                        up_sb[s][32 * g:32 * g + 32, :],
                        start=False, stop=True,
                        skip_group_check=(e > 0),
                        tile_position=(32 * g, 0))
                for mi in range(MCHUNK):
                    m = m0 + mi
                    nc.scalar.activation(
                        asl[e][:, mi * T:(mi + 1) * T], fps[mi],
                        mybir.ActivationFunctionType.Silu,
                        bias=b1c_sb[:, m:m + 1])
            cas = {}
            for e in range(slots):
                s, j = divmod(e, 4)
                ca = cabufs.tile([128, MCHUNK * T], bf16, tag="ca")
                cas[e] = ca
                for mi in range(MCHUNK):
                    nc.vector.tensor_tensor(
                        ca[:, mi * T:(mi + 1) * T],
                        asl[e][:, mi * T:(mi + 1) * T],
                        cbc[:, e * T:(e + 1) * T], op=mybir.AluOpType.mult)
                for mi in range(MCHUNK):
                    m = m0 + mi
                    nc.tensor.matmul(
                        zps[s][32 * j:32 * j + 32, :],
                        a2s_sb[:, m * 256 + s * 128 + 32 * j:m * 256 + s * 128 + 32 * j + 32],
                        ca[:, mi * T:(mi + 1) * T],
                        start=(m == 0), stop=(m == MH - 1),
                        skip_group_check=True,
                        tile_position=(0, 32 * j))
                if e % 2 == 1:      # pairwise DVE reduction tree into abar
                    nc.vector.tensor_tensor(cas[e - 1], cas[e - 1], ca,
                                            op=mybir.AluOpType.add)
            ab_sl = abar[:, m0 * T:(m0 + MCHUNK) * T]
            if slots == 6:
                nc.vector.tensor_tensor(cas[0], cas[0], cas[2], op=mybir.AluOpType.add)
                nc.vector.tensor_tensor(ab_sl, cas[0], cas[4], op=mybir.AluOpType.add)
            elif slots == 8:
                nc.vector.tensor_tensor(cas[0], cas[0], cas[2], op=mybir.AluOpType.add)
                nc.vector.tensor_tensor(cas[4], cas[4], cas[6], op=mybir.AluOpType.add)
                nc.vector.tensor_tensor(ab_sl, cas[0], cas[4], op=mybir.AluOpType.add)
            else:
                acc = cas[0]
                for e in range(2, slots, 2):
                    nc.vector.tensor_tensor(acc, acc, cas[e], op=mybir.AluOpType.add)
                nc.vector.tensor_copy(ab_sl, acc)

        zsb = []
        for s in range(2):
            z_sb = small.tile([128, T], bf16, tag=f"zsb{s}", name=f"zsb{s}")
            na = min(4, max(0, slots - 4 * s))   # active col groups in this stack
            if na < 4:
                nc.vector.memset(z_sb, 0.0)
            if na > 0:
                nc.vector.tensor_copy(z_sb[0:32 * na, :], zps[s][0:32 * na, :])
            zsb.append(z_sb)

        # ---- partial fc2: W2half^T @ abar + B2 lora + b2 ----
        for m2 in range(MD):
            w2m = w2pool.tile([128, MH * 128], bf16, tag="w2m")
            nc.sync.dma_start(w2m, w2p[m2])
            o_ps = psMM.tile([128, T], f32, tag="mm")
            for k2 in range(MH):
                nc.tensor.matmul(o_ps, w2m[:, k2 * 128:(k2 + 1) * 128],
                                 abar[:, k2 * T:(k2 + 1) * T],
                                 start=(k2 == 0), stop=False)
            nc.tensor.matmul(o_ps, b2s_sb[0][:, m2 * 128:(m2 + 1) * 128], zsb[0],
                             start=False, stop=False)
            nc.tensor.matmul(o_ps, b2s_sb[1][:, m2 * 128:(m2 + 1) * 128], zsb[1],
                             start=False, stop=True)
            o_sb = outp.tile([128, T], f32, tag="osb")
            nc.vector.tensor_scalar(o_sb, o_ps, b2c_sb[:, m2:m2 + 1], None,
                                    op0=mybir.AluOpType.add)
            nc.sync.dma_start(outt[:, m2 * T:(m2 + 1) * T], o_sb)

    nc.compile()
    return nc


def _try_balance(req_sets, miss):
    """Exact transportation feasibility via max-flow over eligibility classes.
    Returns per-token quarter assignment or None."""
    from collections import defaultdict
    groups = defaultdict(list)
    for t in range(NT):
        qs = tuple(q for q, mp in enumerate(miss) if not (req_sets[t] & set(mp)))
        if not qs:
            return None
        groups[qs].append(t)
    keys = list(groups)
    # max-flow: source -> class (cap len) -> quarter (cap T) -> sink
    flow = {k: [0] * TQ for k in keys}
    qload = [0] * TQ

    def augment(k):
        # direct
        for q in keys and flow[k] and k:
            pass
        for q in k:
            if qload[q] < T:
                flow[k][q] += 1
                qload[q] += 1
                return True
        # one level of rerouting: move a unit of some other class out of q
        for q in k:
            for k2 in keys:
                if flow[k2][q] > 0:
                    for q2 in k2:
                        if q2 != q and qload[q2] < T:
                            flow[k2][q] -= 1
                            flow[k2][q2] += 1
                            qload[q2] += 1
                            flow[k][q] += 1
                            return True
        # two levels
        for q in k:
            for k2 in keys:
                if flow[k2][q] > 0:
                    for q2 in k2:
                        if q2 == q:
                            continue
                        for k3 in keys:
                            if flow[k3][q2] > 0:
                                for q3 in k3:
                                    if q3 != q2 and qload[q3] < T:
                                        flow[k3][q2] -= 1
                                        flow[k3][q3] += 1
                                        qload[q3] += 1
                                        flow[k2][q] -= 1
                                        flow[k2][q2] += 1
                                        flow[k][q] += 1
                                        return True
        return False

    for k in sorted(keys, key=len):
        for _ in range(len(groups[k])):
            if not augment(k):
                return None
    assign = [-1] * NT
    for k in keys:
        toks = groups[k]
        i = 0
        for q in k:
            for _ in range(flow[k][q]):
                assign[toks[i]] = q
                i += 1
    return assign


def _route_and_balance(x, gate):
    """Host routing + token->quarter assignment. Tries 5-slot quarters
    (missing-triples), then 6-slot (missing-pairs), then dense 8."""
    logits = x.astype(np.float32) @ np.asarray(gate, np.float32).T
    order = np.argsort(-logits, axis=1, kind="stable")
    l = np.take_along_axis(logits, order, axis=1)
    need3 = (l[:, 1] - l[:, 2]) < 1e-3
    req_sets = [set(order[t, :3] if need3[t] else order[t, :2]) for t in range(NT)]

    rng = np.random.RandomState(0)
    for _ in range(60):
        perm8 = rng.permutation(8)
        miss = [set(perm8[0:3]), set(perm8[3:6]),
                set(np.concatenate([perm8[6:8], perm8[0:1]])),
                set(rng.permutation(8)[0:3])]
        miss = [tuple(m) for m in miss]
        # quick pair-coverage check
        ok = all(any(not ({i, j} & set(m)) for m in miss)
                 for i in range(8) for j in range(i + 1, 8))
        if not ok:
            continue
        assign = _try_balance(req_sets, miss)
        if assign is not None:
            perm = np.concatenate(
                [np.where(np.array(assign) == q)[0] for q in range(TQ)])
            slot_experts = [[e for e in range(E) if e not in miss[q]]
                            for q in range(TQ)]
            return perm.astype(np.int64), slot_experts, 5

    miss = [(0, 1), (2, 3), (4, 5), (6, 7)]
    assign = _try_balance(req_sets, miss)
    if assign is not None:
        perm = np.concatenate(
            [np.where(np.array(assign) == q)[0] for q in range(TQ)])
        slot_experts = [[e for e in range(E) if e not in miss[q]]
                        for q in range(TQ)]
        return perm.astype(np.int64), slot_experts, 6

    return np.arange(NT), [list(range(E))] * TQ, 8


def _pack_inputs(hidden_states, gate, W1, b1, W2, b2, A1, B1, A2, B2):
    hs = np.asarray(hidden_states, dtype=np.float32)
    x = hs.reshape(NT, D)
    perm, slot_experts, slots = _route_and_balance(x, gate)
    xT = np.ascontiguousarray(x[perm].T)                 # [D, NT] permuted

    gT = np.asarray(gate, np.float32).T
    gt = np.ascontiguousarray(
        gT.reshape(KD, 128, E).transpose(1, 0, 2).reshape(128, KD * E))

    W1T = np.asarray(W1, np.float32).T                   # [D, H]
    w1p_full = np.ascontiguousarray(
        W1T.reshape(KD, 128, H // 128, 128).transpose(2, 1, 0, 3)
        .reshape(H // 128, 128, KD * 128)).astype(BF)    # [32, 128, 1024]
    W2T = np.asarray(W2, np.float32).T                   # [H, D]
    w2p_full = np.ascontiguousarray(
        W2T.reshape(H // 128, 128, MD, 128).transpose(2, 1, 0, 3)
        .reshape(MD, 128, (H // 128) * 128)).astype(BF)  # [8, 128, 4096]

    A1 = np.asarray(A1, np.float32)
    B1 = np.asarray(B1, np.float32)
    A2 = np.asarray(A2, np.float32)
    B2 = np.asarray(B2, np.float32)

    b1c_full = np.ascontiguousarray(
        np.asarray(b1, np.float32).reshape(H // 128, 128).T)   # [128, 32]
    b2c = np.ascontiguousarray(np.asarray(b2, np.float32).reshape(MD, 128).T)
    b2c_zero = np.zeros_like(b2c)

    # per-quarter slot-permuted stacks
    per_q = []
    for q in range(TQ):
        ex = slot_experts[q]
        S = np.zeros((D, 256), np.float32)
        b1d_full = np.zeros((2, 128, H), np.float32)
        arr = np.zeros((H, 256), np.float32)
        b2sA = np.zeros((2, 128, D), np.float32)
        selA = np.zeros((8, 8 * 128), np.float32)
        for si in range(slots):
            s, g = divmod(si, 4)
            base = s * 128 + 32 * g
            S[:, base:base + 16] = A1[ex[si]].T
            b1d_full[s, 32 * g:32 * g + 16, :] = SC * B1[ex[si]].T
            if si > 0:
                S[:, base + 16:base + 32] = A1[ex[si - 1]].T
                b1d_full[s, 32 * g + 16:32 * g + 32, :] = -SC * B1[ex[si - 1]].T
            arr[:, base:base + 16] = A2[ex[si]].T
            b2sA[s, 32 * g:32 * g + 16, :] = SC * B2[ex[si]].T
            selA[ex[si], si * 128:(si + 1) * 128] = 1.0
        a1s = np.ascontiguousarray(
            S.reshape(KD, 128, 256).transpose(1, 0, 2)
            .reshape(128, KD * 256)).astype(BF)
        a2s_full = np.ascontiguousarray(
            arr.reshape(H // 128, 128, 256).transpose(1, 0, 2)
            .reshape(128, (H // 128) * 256)).astype(BF)
        per_q.append((a1s, b1d_full.astype(BF), a2s_full, b2sA.astype(BF),
                      selA.astype(BF)))

    in_maps = []
    for c in range(NCORES):
        tq, hh = divmod(c, HH)
        a1s, b1d_full, a2s_full, b2sA, selA = per_q[tq]
        xc = xT[:, tq * T:(tq + 1) * T]
        xcp = np.ascontiguousarray(
            xc.reshape(KD, 128, T).transpose(1, 0, 2).reshape(128, KD * T))
        msl = slice(hh * MH, (hh + 1) * MH)
        in_maps.append({
            "xtf": xcp.astype(np.float32),
            "xtb": xcp.astype(BF),
            "gt": gt,
            "w1p": np.ascontiguousarray(w1p_full[msl]),
            "w2p": np.ascontiguousarray(w2p_full[:, :, hh * MH * 128:(hh + 1) * MH * 128]),
            "a1s": a1s,
            "b1d": np.ascontiguousarray(b1d_full[:, :, hh * HL:(hh + 1) * HL]),
            "a2s": np.ascontiguousarray(a2s_full[:, hh * MH * 256:(hh + 1) * MH * 256]),
            "b2s": b2sA,
            "b1c": np.ascontiguousarray(b1c_full[:, msl]),
            "b2c": b2c if hh == 0 else b2c_zero,
            "sel": selA,
        })
    return in_maps, perm, slots


_NC_CACHE = {}


def get_nc(slots=8):
    if slots not in _NC_CACHE:
        _NC_CACHE[slots] = _build_bass(slots)
    return _NC_CACHE[slots]


def _unpack_outputs(results, perm):
    cols = []
    for tq in range(TQ):
        o = None
        for hh in range(HH):
            c = tq * HH + hh
            p = np.asarray(results[c]["outt"], np.float32)
            p = p.reshape(128, MD, T).transpose(1, 0, 2).reshape(D, T)
            o = p if o is None else o + p
        cols.append(o)
    outT = np.concatenate(cols, axis=1)                  # [D, NT] (permuted tokens)
    out = np.empty((NT, D), np.float32)
    out[perm] = outT.T
    return out.reshape(2, NT // 2, D)


def kernel(**inputs):
    in_maps, perm, slots = _pack_inputs(**inputs)
    nc = get_nc(slots)
    res = run_bass_kernel_spmd(nc, in_maps, core_ids=list(range(NCORES)))
    return _unpack_outputs(res.results, perm)



# revision 3
# speedup vs baseline: 1.5568x; 1.5568x over previous
"""Trainium2 Bass kernel for MixLoRA sparse MoE (8 experts, top-2, shared base MLP).

Dense-mask formulation (no per-slot loops, no balancing): with top-2 routing,
  abar = wa*silu(common + d1) + wb*silu(common + d2),
  d_j  = B1cat_sc^T @ (mjx . u),   u = A1cat @ x  (all 8 experts stacked: 8*16=128 rows)
where m1x/m2x are host-shipped one-hot masks in (expert,rank)-row space and
wa = sigmoid(l_top1 - l_top2) (device-computed logits), wb = 1-wa. fc2 uses
  out = W2^T @ abar + B2cat_sc^T @ (m1x.z1 + m2x.z2) + b2,
  z_j = A2cat @ (w_j * a_j).
Sharding: 4 token quarters (T=512) x 2 H-halves (HL=2048); the host sums the
H-pair partials. All heavy matmuls are bf16 K=128 N=512; router fp32.
"""

import sys, os
sys.path.insert(0, "/opt/trn_rl_repo")

from contextlib import ExitStack

import numpy as np
import ml_dtypes

import concourse.bass as bass
import concourse.tile as tile
from concourse import mybir, bacc
from concourse.bass_utils import run_bass_kernel_spmd

BF = ml_dtypes.bfloat16

NCORES = 8
TQ = 4               # token shards
HH = 2               # H shards
D, H, E, R = 1024, 4096, 8, 16
NT = 2048
T = NT // TQ         # tokens per core (512)
HL = H // HH         # H per core (2048)
KD = D // 128        # 8
MH = HL // 128       # 16 local H slices
MD = D // 128        # 8
SC = 2.0

f32 = mybir.dt.float32
bf16 = mybir.dt.bfloat16


def _build_bass(slots=0):
    nc = bacc.Bacc("TRN2", target_bir_lowering=False, debug=False)

    xtb = nc.dram_tensor("xtb", [128, KD * T], bf16, kind="ExternalInput")
    gt = nc.dram_tensor("gt", [128, KD * E], bf16, kind="ExternalInput")
    dm8 = nc.dram_tensor("dm8", [8, T], f32, kind="ExternalInput")
    ones8c = nc.dram_tensor("ones8c", [8, 128], f32, kind="ExternalInput")
    m1x = nc.dram_tensor("m1x", [128, T], bf16, kind="ExternalInput")
    m2x = nc.dram_tensor("m2x", [128, T], bf16, kind="ExternalInput")
    a1p = nc.dram_tensor("a1p", [128, KD * 128], bf16, kind="ExternalInput")
    b1p = nc.dram_tensor("b1p", [128, MH * 128], bf16, kind="ExternalInput")
    a2p = nc.dram_tensor("a2p", [128, MH * 128], bf16, kind="ExternalInput")
    b2p = nc.dram_tensor("b2p", [128, MD * 128], bf16, kind="ExternalInput")
    w1p = nc.dram_tensor("w1p", [MH, 128, KD * 128], bf16, kind="ExternalInput")
    w2p = nc.dram_tensor("w2p", [MD, 128, MH * 128], bf16, kind="ExternalInput")
    b1c = nc.dram_tensor("b1c", [128, MH], f32, kind="ExternalInput")
    b2c = nc.dram_tensor("b2c", [128, MD], f32, kind="ExternalInput")
    outt = nc.dram_tensor("outt", [128, MD * T], f32, kind="ExternalOutput")

    with tile.TileContext(nc) as tc, ExitStack() as ctx:
        consts = ctx.enter_context(tc.tile_pool(name="consts", bufs=1))
        wpool = ctx.enter_context(tc.tile_pool(name="wpool", bufs=4))
        w2pool = ctx.enter_context(tc.tile_pool(name="w2pool", bufs=3))
        abufs = ctx.enter_context(tc.tile_pool(name="abufs", bufs=4))
        outp = ctx.enter_context(tc.tile_pool(name="outp", bufs=3))
        psF = ctx.enter_context(tc.tile_pool(name="psF", bufs=2, space="PSUM"))
        psZ = ctx.enter_context(tc.tile_pool(name="psZ", bufs=1, space="PSUM"))
        psR = ctx.enter_context(tc.tile_pool(name="psR", bufs=1, space="PSUM"))

        xtb_sb = consts.tile([128, KD * T], bf16, tag="xtb_sb")
        for k in range(KD):
            nc.sync.dma_start(xtb_sb[:, k * T:(k + 1) * T], xtb[:, k * T:(k + 1) * T])
        gt_sb = consts.tile([128, KD * E], bf16, tag="gt_sb")
        nc.sync.dma_start(gt_sb, gt[:])
        dm8_sb = consts.tile([8, T], f32, tag="dm8_sb")
        nc.sync.dma_start(dm8_sb, dm8[:])
        ones8c_sb = consts.tile([8, 128], f32, tag="ones8c_sb")
        nc.sync.dma_start(ones8c_sb, ones8c[:])
        m1x_sb = consts.tile([128, T], bf16, tag="m1x_sb")
        nc.sync.dma_start(m1x_sb, m1x[:])
        m2x_sb = consts.tile([128, T], bf16, tag="m2x_sb")
        nc.sync.dma_start(m2x_sb, m2x[:])
        a1p_sb = consts.tile([128, KD * 128], bf16, tag="a1p_sb")
        nc.sync.dma_start(a1p_sb, a1p[:])
        b1p_sb = consts.tile([128, MH * 128], bf16, tag="b1p_sb")
        nc.sync.dma_start(b1p_sb, b1p[:])
        a2p_sb = consts.tile([128, MH * 128], bf16, tag="a2p_sb")
        nc.sync.dma_start(a2p_sb, a2p[:])
        b2p_sb = consts.tile([128, MD * 128], bf16, tag="b2p_sb")
        nc.sync.dma_start(b2p_sb, b2p[:])
        b1c_sb = consts.tile([128, MH], f32, tag="b1c_sb")
        nc.sync.dma_start(b1c_sb, b1c[:])
        b2c_sb = consts.tile([128, MD], f32, tag="b2c_sb")
        nc.sync.dma_start(b2c_sb, b2c[:])

        def xtb_k(k):
            return xtb_sb[:, k * T:(k + 1) * T]

        # ---- Router: lg[8,T] -> l12 broadcast to 128 partitions -> sigmoid ----
        lg_ps = psR.tile([8, T], f32, tag="lg", name="lg_ps")
        for k in range(KD):
            nc.tensor.matmul(lg_ps, gt_sb[:, k * E:(k + 1) * E], xtb_k(k),
                             start=(k == 0), stop=(k == KD - 1))
        mlg_sb = consts.tile([8, T], f32, tag="mlg_sb")
        nc.vector.tensor_tensor(mlg_sb, lg_ps, dm8_sb, op=mybir.AluOpType.mult)
        wb_ps = psR.tile([128, T], f32, tag="wb", name="wb_ps")
        nc.tensor.matmul(wb_ps, ones8c_sb, mlg_sb, start=True, stop=True)
        wab_sb = consts.tile([128, T], bf16, tag="wab_sb")
        nc.scalar.activation(wab_sb, wb_ps, mybir.ActivationFunctionType.Sigmoid)
        wbb_sb = consts.tile([128, T], bf16, tag="wbb_sb")
        nc.vector.tensor_scalar(wbb_sb, wab_sb, -1.0, 1.0,
                                op0=mybir.AluOpType.mult,
                                op1=mybir.AluOpType.add)

        # ---- u = A1cat @ x; masked cu1/cu2; cud = cu2-cu1 ----
        u_ps = psR.tile([128, T], f32, tag="u", name="u_ps")
        for k in range(KD):
            nc.tensor.matmul(u_ps, a1p_sb[:, k * 128:(k + 1) * 128], xtb_k(k),
                             start=(k == 0), stop=(k == KD - 1))
        cu1_sb = consts.tile([128, T], bf16, tag="cu1_sb")
        nc.vector.tensor_tensor(cu1_sb, u_ps, m1x_sb, op=mybir.AluOpType.mult)
        cu2_sb = consts.tile([128, T], bf16, tag="cu2_sb")
        nc.vector.tensor_tensor(cu2_sb, u_ps, m2x_sb, op=mybir.AluOpType.mult)
        cud_sb = consts.tile([128, T], bf16, tag="cud_sb")
        nc.vector.tensor_tensor(cud_sb, cu2_sb, cu1_sb, op=mybir.AluOpType.subtract)

        # ---- fc1 m-loop: common+d1 -> silu -> aw1; +(d2-d1) -> silu -> aw2 ----
        aw1_all = consts.tile([128, MH * T], bf16, tag="aw1_all")
        aw2_all = consts.tile([128, MH * T], bf16, tag="aw2_all")
        abar_all = consts.tile([128, MH * T], bf16, tag="abar_all")
        z1_ps = psZ.tile([128, T], f32, tag="z1", name="z1_ps")
        z2_ps = psZ.tile([128, T], f32, tag="z2", name="z2_ps")
        for m in range(MH):
            msl = slice(m * T, (m + 1) * T)
            w1m = wpool.tile([128, KD * 128], bf16, tag="w1m", name="w1m")
            nc.sync.dma_start(w1m, w1p[m])
            f_ps = psF.tile([128, T], f32, tag="mm", name="f_ps")
            for k in range(KD):
                nc.tensor.matmul(f_ps, w1m[:, k * 128:(k + 1) * 128], xtb_k(k),
                                 start=(k == 0), stop=False)
            nc.tensor.matmul(f_ps, b1p_sb[:, m * 128:(m + 1) * 128], cu1_sb,
                             start=False, stop=True)
            a1t = abufs.tile([128, T], bf16, tag="a1t", name="a1t")
            nc.scalar.activation(a1t, f_ps, mybir.ActivationFunctionType.Silu,
                                 bias=b1c_sb[:, m:m + 1])
            nc.vector.tensor_tensor(aw1_all[:, msl], a1t, wab_sb,
                                    op=mybir.AluOpType.mult)
            nc.tensor.matmul(f_ps, b1p_sb[:, m * 128:(m + 1) * 128], cud_sb,
                             start=False, stop=True, skip_group_check=True)
            a2t = abufs.tile([128, T], bf16, tag="a2t", name="a2t")
            nc.scalar.activation(a2t, f_ps, mybir.ActivationFunctionType.Silu,
                                 bias=b1c_sb[:, m:m + 1])
            nc.vector.tensor_tensor(aw2_all[:, msl], a2t, wbb_sb,
                                    op=mybir.AluOpType.mult)
            nc.tensor.matmul(z1_ps, a2p_sb[:, m * 128:(m + 1) * 128],
                             aw1_all[:, msl], start=(m == 0), stop=(m == MH - 1))
            nc.tensor.matmul(z2_ps, a2p_sb[:, m * 128:(m + 1) * 128],
                             aw2_all[:, msl], start=(m == 0), stop=(m == MH - 1))
            nc.gpsimd.tensor_tensor(abar_all[:, msl], aw1_all[:, msl],
                                    aw2_all[:, msl], op=mybir.AluOpType.add)

        # ---- v2 = m1x.z1 + m2x.z2 ----
        zt1 = consts.tile([128, T], bf16, tag="zt1")
        nc.vector.tensor_tensor(zt1, z1_ps, m1x_sb, op=mybir.AluOpType.mult)
        zt2 = consts.tile([128, T], bf16, tag="zt2")
        nc.vector.tensor_tensor(zt2, z2_ps, m2x_sb, op=mybir.AluOpType.mult)
        v2_sb = consts.tile([128, T], bf16, tag="v2_sb")
        nc.vector.tensor_tensor(v2_sb, zt1, zt2, op=mybir.AluOpType.add)

        # ---- fc2: W2half^T @ abar + B2cat_sc^T @ v2 (+ b2 on hh==0) ----
        for m2 in range(MD):
            w2m = w2pool.tile([128, MH * 128], bf16, tag="w2m")
            nc.sync.dma_start(w2m, w2p[m2])
            o_ps = psF.tile([128, T], f32, tag="mm")
            for k2 in range(MH):
                nc.tensor.matmul(o_ps, w2m[:, k2 * 128:(k2 + 1) * 128],
                                 abar_all[:, k2 * T:(k2 + 1) * T],
                                 start=(k2 == 0), stop=False)
            nc.tensor.matmul(o_ps, b2p_sb[:, m2 * 128:(m2 + 1) * 128], v2_sb,
                             start=False, stop=True)
            o_sb = outp.tile([128, T], f32, tag="osb")
            nc.vector.tensor_scalar(o_sb, o_ps, b2c_sb[:, m2:m2 + 1], None,
                                    op0=mybir.AluOpType.add)
            nc.sync.dma_start(outt[:, m2 * T:(m2 + 1) * T], o_sb)

    nc.compile()
    return nc


def _pack_inputs(hidden_states, gate, W1, b1, W2, b2, A1, B1, A2, B2):
    hs = np.asarray(hidden_states, dtype=np.float32)
    x = hs.reshape(NT, D)
    gate = np.asarray(gate, np.float32)

    # Host routing: top-2 selection masks only (weights computed on device).
    logits = x @ gate.T
    order = np.argsort(-logits, axis=1, kind="stable")
    m1 = np.zeros((NT, E), np.float32)
    m2 = np.zeros((NT, E), np.float32)
    np.put_along_axis(m1, order[:, :1], 1.0, axis=1)
    np.put_along_axis(m2, order[:, 1:2], 1.0, axis=1)
    m1xf = np.repeat(m1, R, axis=1)          # [NT, 128]
    m2xf = np.repeat(m2, R, axis=1)
    dm8f = (m1 - m2).T                       # [E, NT]

    xT = np.ascontiguousarray(x.T)           # [D, NT]

    gT = gate.T
    gt = np.ascontiguousarray(
        gT.reshape(KD, 128, E).transpose(1, 0, 2).reshape(128, KD * E)).astype(BF)

    W1T = np.asarray(W1, np.float32).T       # [D, H]
    w1p_full = np.ascontiguousarray(
        W1T.reshape(KD, 128, H // 128, 128).transpose(2, 1, 0, 3)
        .reshape(H // 128, 128, KD * 128)).astype(BF)
    W2T = np.asarray(W2, np.float32).T       # [H, D]
    w2p_full = np.ascontiguousarray(
        W2T.reshape(H // 128, 128, MD, 128).transpose(2, 1, 0, 3)
        .reshape(MD, 128, (H // 128) * 128)).astype(BF)

    A1 = np.asarray(A1, np.float32)
    B1 = np.asarray(B1, np.float32)
    A2 = np.asarray(A2, np.float32)
    B2 = np.asarray(B2, np.float32)

    A1cat = A1.reshape(E * R, D)                                   # [128, D]
    a1p = np.ascontiguousarray(
        A1cat.T.reshape(KD, 128, 128).transpose(1, 0, 2)
        .reshape(128, KD * 128)).astype(BF)
    B1cat = SC * np.concatenate([B1[e] for e in range(E)], axis=1)  # [H, 128]
    A2cat = A2.reshape(E * R, H)                                    # [128, H]
    B2cat = SC * np.concatenate([B2[e] for e in range(E)], axis=1)  # [D, 128]
    # b2p[m2]: lhsT = B2cat[m2-tile rows].T -> [128(er), 128(D-cols)]
    b2p = np.ascontiguousarray(
        B2cat.reshape(MD, 128, 128).transpose(2, 0, 1)
        .reshape(128, MD * 128)).astype(BF)

    b1c_full = np.ascontiguousarray(
        np.asarray(b1, np.float32).reshape(H // 128, 128).T)        # [128, 32]
    b2c = np.ascontiguousarray(np.asarray(b2, np.float32).reshape(MD, 128).T)
    b2c_zero = np.zeros_like(b2c)
    ones8c = np.ones((8, 128), np.float32)

    in_maps = []
    for c in range(NCORES):
        tq, hh = divmod(c, HH)
        tsl = slice(tq * T, (tq + 1) * T)
        xc = xT[:, tsl]
        xcp = np.ascontiguousarray(
            xc.reshape(KD, 128, T).transpose(1, 0, 2).reshape(128, KD * T))
        hsl = slice(hh * HL, (hh + 1) * HL)
        msl = slice(hh * MH, (hh + 1) * MH)
        # b1p[m]: lhsT = B1cat[hh-local m-tile rows].T -> [128(er), 128(H-cols)]
        b1ph = np.ascontiguousarray(
            B1cat[hsl].reshape(MH, 128, 128).transpose(2, 0, 1)
            .reshape(128, MH * 128)).astype(BF)
        # a2p[m]: lhsT = A2cat[:, hh-local m-tile].T -> [128(H-rows), 128(er)]
        a2ph = np.ascontiguousarray(
            A2cat[:, hsl].T.reshape(MH, 128, 128).transpose(1, 0, 2)
            .reshape(128, MH * 128)).astype(BF)
        in_maps.append({
            "xtb": xcp.astype(BF),
            "gt": gt,
            "dm8": np.ascontiguousarray(dm8f[:, tsl]),
            "ones8c": ones8c,
            "m1x": np.ascontiguousarray(m1xf[tsl].T).astype(BF),
            "m2x": np.ascontiguousarray(m2xf[tsl].T).astype(BF),
            "a1p": a1p,
            "b1p": b1ph,
            "a2p": a2ph,
            "b2p": b2p,
            "w1p": np.ascontiguousarray(w1p_full[msl]),
            "w2p": np.ascontiguousarray(
                w2p_full[:, :, hh * MH * 128:(hh + 1) * MH * 128]),
            "b1c": np.ascontiguousarray(b1c_full[:, msl]),
            "b2c": b2c if hh == 0 else b2c_zero,
        })
    return in_maps, np.arange(NT), 0


_NC_CACHE = {}


def get_nc(slots=0):
    if slots not in _NC_CACHE:
        _NC_CACHE[slots] = _build_bass(slots)
    return _NC_CACHE[slots]


def _unpack_outputs(results, perm):
    cols = []
    for tq in range(TQ):
        o = None
        for hh in range(HH):
            c = tq * HH + hh
            p = np.asarray(results[c]["outt"], np.float32)
            p = p.reshape(128, MD, T).transpose(1, 0, 2).reshape(D, T)
            o = p if o is None else o + p
        cols.append(o)
    outT = np.concatenate(cols, axis=1)                  # [D, NT]
    out = np.empty((NT, D), np.float32)
    out[perm] = outT.T
    return out.reshape(2, NT // 2, D)


def kernel(**inputs):
    in_maps, perm, slots = _pack_inputs(**inputs)
    nc = get_nc(slots)
    res = run_bass_kernel_spmd(nc, in_maps, core_ids=list(range(NCORES)))
    return _unpack_outputs(res.results, perm)


# revision 7
# speedup vs baseline: 1.9881x; 1.2771x over previous
"""Trainium2 Bass kernel for MixLoRA sparse MoE (8 experts, top-2, shared base MLP).

Dense-mask formulation (no per-slot loops, no balancing): with top-2 routing,
  abar = wa*silu(common + d1) + wb*silu(common + d2),
  d_j  = B1cat_sc^T @ (mjx . u),   u = A1cat @ x  (all 8 experts stacked: 8*16=128 rows)
where m1x/m2x are host-shipped one-hot masks in (expert,rank)-row space and
wa = sigmoid(l_top1 - l_top2) (device-computed logits), wb = 1-wa. fc2 uses
  out = W2^T @ abar + B2cat_sc^T @ (m1x.z1 + m2x.z2) + b2,
  z_j = A2cat @ (w_j * a_j).
Sharding: 4 token quarters (T=512) x 2 H-halves (HL=2048); the host sums the
H-pair partials. All heavy matmuls are bf16 K=128 N=512; router fp32.
Perf structure: warmup matmuls cover the DMA head (HAM warm before real MMs);
bf16 consts packed into one dram tensor loaded by 3 priority-ordered DMAs;
W1 prefetched in 4-tile chunks; W2 loaded mid-kernel via ScalarE's DGE ring;
output DMAs also on ScalarE to keep SyncE's ring free.
"""

import sys, os
sys.path.insert(0, "/opt/trn_rl_repo")

from contextlib import ExitStack

import numpy as np
import ml_dtypes

import concourse.bass as bass
import concourse.tile as tile
from concourse import mybir, bacc
from concourse.bass_utils import run_bass_kernel_spmd

BF = ml_dtypes.bfloat16

NCORES = 8
TQ = 4               # token shards
HH = 2               # H shards
D, H, E, R = 1024, 4096, 8, 16
NT = 2048
T = NT // TQ         # tokens per core (512)
HL = H // HH         # H per core (2048)
KD = D // 128        # 8
MH = HL // 128       # 16 local H slices
MD = D // 128        # 8
SC = 2.0
NWARM = 16

f32 = mybir.dt.float32
bf16 = mybir.dt.bfloat16

# column offsets in the packed bf16 consts tensor
C_XTB = 0                    # [128, KD*T]  x k-tiled            (4096)
C_GT = C_XTB + KD * T        # [128, KD*E]  gate.T k-tiled       (64)
C_A1P = C_GT + KD * E        # [128, KD*128] A1cat.T k-tiled     (1024)
C_M1X = C_A1P + KD * 128     # [128, T]                          (512)
C_M2X = C_M1X + T            # [128, T]                          (512)
C_B1P = C_M2X + T            # [128, MH*128] B1cat_sc lhsT       (2048)
C_A2P = C_B1P + MH * 128     # [128, MH*128] A2cat.T lhsT        (2048)
C_B2P = C_A2P + MH * 128     # [128, MD*128] B2cat_sc lhsT       (1024)
C_END = C_B2P + MD * 128
G1_END = C_M1X               # dma group 1: xtb+gt+a1p
G2_END = C_A2P               # dma group 2: m1x,m2x,b1p
G3_END = C_END               # dma group 3: a2p,b2p

W1CH = 4                     # w1 m-tiles per dma chunk
W2CH = 4                     # w2 m2-tiles per dma chunk


def _build_bass(slots=0):
    nc = bacc.Bacc("TRN2", target_bir_lowering=False, debug=False)

    bigc = nc.dram_tensor("bigc", [128, C_END], bf16, kind="ExternalInput")
    sm8 = nc.dram_tensor("sm8", [8, T + 128], f32, kind="ExternalInput")
    smc = nc.dram_tensor("smc", [128, MH + MD], f32, kind="ExternalInput")
    w1p = nc.dram_tensor("w1p", [MH // W1CH, 128, W1CH * KD * 128], bf16,
                         kind="ExternalInput")
    w2p = nc.dram_tensor("w2p", [MD // W2CH, 128, W2CH * MH * 128], bf16,
                         kind="ExternalInput")
    outt = nc.dram_tensor("outt", [128, MD * T], f32, kind="ExternalOutput")

    with tile.TileContext(nc) as tc, ExitStack() as ctx:
        consts = ctx.enter_context(tc.tile_pool(name="consts", bufs=1))
        wpool = ctx.enter_context(tc.tile_pool(name="wpool", bufs=2))
        abufs = ctx.enter_context(tc.tile_pool(name="abufs", bufs=4))
        outp = ctx.enter_context(tc.tile_pool(name="outp", bufs=3))
        psF = ctx.enter_context(tc.tile_pool(name="psF", bufs=2, space="PSUM"))
        psZ = ctx.enter_context(tc.tile_pool(name="psZ", bufs=1, space="PSUM"))
        psR = ctx.enter_context(tc.tile_pool(name="psR", bufs=1, space="PSUM"))

        # ---- warmup: keep PE busy (and HAM warm) during the DMA head ----
        warm_w = consts.tile([128, 128], bf16, tag="warm_w")
        nc.vector.memset(warm_w, 0.0)
        warm_x = consts.tile([128, T], bf16, tag="warm_x")
        nc.vector.memset(warm_x, 0.0)
        warm_ps = psR.tile([128, T], f32, tag="warm", name="warm_ps")
        for i in range(NWARM):
            nc.tensor.matmul(warm_ps, warm_w, warm_x, start=True, stop=True)

        # ---- input DMAs, priority order ----
        bigc_sb = consts.tile([128, C_END], bf16, tag="bigc_sb")
        nc.sync.dma_start(bigc_sb[:, :G1_END], bigc[:, :G1_END])
        sm8_sb = consts.tile([8, T + 128], f32, tag="sm8_sb")
        nc.sync.dma_start(sm8_sb, sm8[:])
        smc_sb = consts.tile([128, MH + MD], f32, tag="smc_sb")
        nc.sync.dma_start(smc_sb, smc[:])
        w1t = [None] * (MH // W1CH)
        w1t[0] = wpool.tile([128, W1CH * KD * 128], bf16, tag="w1t", name="w1t0")
        nc.sync.dma_start(w1t[0], w1p[0])
        nc.sync.dma_start(bigc_sb[:, G1_END:G2_END], bigc[:, G1_END:G2_END])
        nc.sync.dma_start(bigc_sb[:, G2_END:G3_END], bigc[:, G2_END:G3_END])
        w1t[1] = wpool.tile([128, W1CH * KD * 128], bf16, tag="w1t", name="w1t1")
        nc.sync.dma_start(w1t[1], w1p[1])

        def bc(c0, n):
            return bigc_sb[:, c0:c0 + n]

        def xtb_k(k):
            return bigc_sb[:, C_XTB + k * T:C_XTB + (k + 1) * T]

        dm8_sb = sm8_sb[:, :T]
        ones8c_sb = sm8_sb[:, T:]
        b1c_sb = smc_sb[:, :MH]
        b2c_sb = smc_sb[:, MH:]

        # ---- Router: lg[8,T] -> l12 broadcast to 128 partitions -> sigmoid ----
        lg_ps = psR.tile([8, T], f32, tag="lg", name="lg_ps")
        for k in range(KD):
            nc.tensor.matmul(lg_ps, bc(C_GT + k * E, E), xtb_k(k),
                             start=(k == 0), stop=(k == KD - 1))
        mlg_sb = consts.tile([8, T], f32, tag="mlg_sb")
        nc.vector.tensor_tensor(mlg_sb, lg_ps, dm8_sb, op=mybir.AluOpType.mult)
        wb_ps = psR.tile([128, T], f32, tag="wb", name="wb_ps")
        nc.tensor.matmul(wb_ps, ones8c_sb, mlg_sb, start=True, stop=True)
        wab_sb = consts.tile([128, T], bf16, tag="wab_sb")
        nc.scalar.activation(wab_sb, wb_ps, mybir.ActivationFunctionType.Sigmoid)
        wbb_sb = consts.tile([128, T], bf16, tag="wbb_sb")
        nc.vector.tensor_scalar(wbb_sb, wab_sb, -1.0, 1.0,
                                op0=mybir.AluOpType.mult,
                                op1=mybir.AluOpType.add)

        # ---- u = A1cat @ x; masked cu1/cu2; cud = cu2-cu1 ----
        u_ps = psR.tile([128, T], f32, tag="u", name="u_ps")
        for k in range(KD):
            nc.tensor.matmul(u_ps, bc(C_A1P + k * 128, 128), xtb_k(k),
                             start=(k == 0), stop=(k == KD - 1))
        cu1_sb = consts.tile([128, T], bf16, tag="cu1_sb")
        nc.vector.tensor_tensor(cu1_sb, u_ps, bc(C_M1X, T), op=mybir.AluOpType.mult)
        cu2_sb = consts.tile([128, T], bf16, tag="cu2_sb")
        nc.vector.tensor_tensor(cu2_sb, u_ps, bc(C_M2X, T), op=mybir.AluOpType.mult)
        cud_sb = consts.tile([128, T], bf16, tag="cud_sb")
        nc.vector.tensor_tensor(cud_sb, cu2_sb, cu1_sb, op=mybir.AluOpType.subtract)

        # ---- fc1 m-loop: common+d1 -> silu -> aw1; +(d2-d1) -> silu -> aw2 ----
        aw1_all = consts.tile([128, MH * T], bf16, tag="aw1_all")
        aw2_all = consts.tile([128, MH * T], bf16, tag="aw2_all")
        abar_all = consts.tile([128, MH * T], bf16, tag="abar_all")
        w2_sb = consts.tile([128, MD * MH * 128], bf16, tag="w2_sb")
        z1_ps = psZ.tile([128, T], f32, tag="z1", name="z1_ps")
        z2_ps = psZ.tile([128, T], f32, tag="z2", name="z2_ps")
        for m in range(MH):
            msl = slice(m * T, (m + 1) * T)
            ch, mi = divmod(m, W1CH)
            if mi == 2 and ch + 2 < MH // W1CH:  # prefetch chunk ch+2 (ring of 2)
                w1t[ch + 2] = wpool.tile([128, W1CH * KD * 128], bf16, tag="w1t",
                                         name=f"w1t{ch + 2}")
                nc.sync.dma_start(w1t[ch + 2], w1p[ch + 2])
            if m == 4:               # mid-kernel W2 loads on ScalarE's DGE ring
                nc.sync.dma_start(w2_sb[:, :MD * MH * 64], w2p[0])
            if m == 8:
                nc.sync.dma_start(w2_sb[:, MD * MH * 64:], w2p[1])
            w1m = w1t[ch]
            f_ps = psF.tile([128, T], f32, tag="mm", name="f_ps")
            for k in range(KD):
                nc.tensor.matmul(f_ps, w1m[:, (mi * KD + k) * 128:(mi * KD + k + 1) * 128],
                                 xtb_k(k), start=(k == 0), stop=False)
            nc.tensor.matmul(f_ps, bc(C_B1P + m * 128, 128), cu1_sb,
                             start=False, stop=True)
            a1t = abufs.tile([128, T], bf16, tag="a1t", name="a1t")
            nc.scalar.activation(a1t, f_ps, mybir.ActivationFunctionType.Silu,
                                 bias=b1c_sb[:, m:m + 1])
            nc.vector.tensor_tensor(aw1_all[:, msl], a1t, wab_sb,
                                    op=mybir.AluOpType.mult)
            nc.tensor.matmul(f_ps, bc(C_B1P + m * 128, 128), cud_sb,
                             start=False, stop=True, skip_group_check=True)
            a2t = abufs.tile([128, T], bf16, tag="a2t", name="a2t")
            nc.scalar.activation(a2t, f_ps, mybir.ActivationFunctionType.Silu,
                                 bias=b1c_sb[:, m:m + 1])
            nc.vector.tensor_tensor(aw2_all[:, msl], a2t, wbb_sb,
                                    op=mybir.AluOpType.mult)
            nc.tensor.matmul(z1_ps, bc(C_A2P + m * 128, 128),
                             aw1_all[:, msl], start=(m == 0), stop=(m == MH - 1))
            nc.tensor.matmul(z2_ps, bc(C_A2P + m * 128, 128),
                             aw2_all[:, msl], start=(m == 0), stop=(m == MH - 1))
            nc.gpsimd.tensor_tensor(abar_all[:, msl], aw1_all[:, msl],
                                    aw2_all[:, msl], op=mybir.AluOpType.add)

        # ---- v2 = m1x.z1 + m2x.z2 ----
        zt1 = consts.tile([128, T], bf16, tag="zt1")
        nc.vector.tensor_tensor(zt1, z1_ps, bc(C_M1X, T), op=mybir.AluOpType.mult)
        zt2 = consts.tile([128, T], bf16, tag="zt2")
        nc.vector.tensor_tensor(zt2, z2_ps, bc(C_M2X, T), op=mybir.AluOpType.mult)
        v2_sb = consts.tile([128, T], bf16, tag="v2_sb")
        nc.vector.tensor_tensor(v2_sb, zt1, zt2, op=mybir.AluOpType.add)

        # ---- fc2: W2half^T @ abar + B2cat_sc^T @ v2 (+ b2 on hh==0) ----
        for m2 in range(MD):
            o_ps = psF.tile([128, T], f32, tag="mm")
            for k2 in range(MH):
                nc.tensor.matmul(o_ps, w2_sb[:, (m2 * MH + k2) * 128:(m2 * MH + k2 + 1) * 128],
                                 abar_all[:, k2 * T:(k2 + 1) * T],
                                 start=(k2 == 0), stop=False)
            nc.tensor.matmul(o_ps, bc(C_B2P + m2 * 128, 128), v2_sb,
                             start=False, stop=True)
            o_sb = outp.tile([128, T], f32, tag="osb")
            nc.vector.tensor_scalar(o_sb, o_ps, b2c_sb[:, m2:m2 + 1], None,
                                    op0=mybir.AluOpType.add)
            nc.sync.dma_start(outt[:, m2 * T:(m2 + 1) * T], o_sb)

    nc.compile()
    return nc


def _pack_inputs(hidden_states, gate, W1, b1, W2, b2, A1, B1, A2, B2):
    hs = np.asarray(hidden_states, dtype=np.float32)
    x = hs.reshape(NT, D)
    gate = np.asarray(gate, np.float32)

    # Host routing: top-2 selection masks only (weights computed on device).
    logits = x @ gate.T
    order = np.argsort(-logits, axis=1, kind="stable")
    m1 = np.zeros((NT, E), np.float32)
    m2 = np.zeros((NT, E), np.float32)
    np.put_along_axis(m1, order[:, :1], 1.0, axis=1)
    np.put_along_axis(m2, order[:, 1:2], 1.0, axis=1)
    m1xf = np.repeat(m1, R, axis=1)          # [NT, 128]
    m2xf = np.repeat(m2, R, axis=1)
    dm8f = (m1 - m2).T                       # [E, NT]

    xT = np.ascontiguousarray(x.T)           # [D, NT]

    gT = gate.T
    gt = np.ascontiguousarray(
        gT.reshape(KD, 128, E).transpose(1, 0, 2).reshape(128, KD * E)).astype(BF)

    W1T = np.asarray(W1, np.float32).T       # [D, H]
    w1p_full = np.ascontiguousarray(
        W1T.reshape(KD, 128, H // 128, 128).transpose(2, 1, 0, 3)
        .reshape(H // 128, 128, KD * 128)).astype(BF)
    W2T = np.asarray(W2, np.float32).T       # [H, D]
    w2p_full = np.ascontiguousarray(
        W2T.reshape(H // 128, 128, MD, 128).transpose(2, 1, 0, 3)
        .reshape(MD, 128, (H // 128) * 128)).astype(BF)

    A1 = np.asarray(A1, np.float32)
    B1 = np.asarray(B1, np.float32)
    A2 = np.asarray(A2, np.float32)
    B2 = np.asarray(B2, np.float32)

    A1cat = A1.reshape(E * R, D)                                    # [128, D]
    a1p = np.ascontiguousarray(
        A1cat.T.reshape(KD, 128, 128).transpose(1, 0, 2)
        .reshape(128, KD * 128)).astype(BF)
    B1cat = SC * np.concatenate([B1[e] for e in range(E)], axis=1)  # [H, 128]
    A2cat = A2.reshape(E * R, H)                                    # [128, H]
    B2cat = SC * np.concatenate([B2[e] for e in range(E)], axis=1)  # [D, 128]
    # b2p[m2]: lhsT = B2cat[m2-tile rows].T -> [128(er), 128(D-cols)]
    b2p = np.ascontiguousarray(
        B2cat.reshape(MD, 128, 128).transpose(2, 0, 1)
        .reshape(128, MD * 128)).astype(BF)

    b1c_full = np.ascontiguousarray(
        np.asarray(b1, np.float32).reshape(H // 128, 128).T)        # [128, 32]
    b2c = np.ascontiguousarray(np.asarray(b2, np.float32).reshape(MD, 128).T)
    b2c_zero = np.zeros_like(b2c)

    in_maps = []
    for c in range(NCORES):
        tq, hh = divmod(c, HH)
        tsl = slice(tq * T, (tq + 1) * T)
        xc = xT[:, tsl]
        xcp = np.ascontiguousarray(
            xc.reshape(KD, 128, T).transpose(1, 0, 2).reshape(128, KD * T))
        hsl = slice(hh * HL, (hh + 1) * HL)
        msl = slice(hh * MH, (hh + 1) * MH)
        # b1p[m]: lhsT = B1cat[hh-local m-tile rows].T -> [128(er), 128(H-cols)]
        b1ph = np.ascontiguousarray(
            B1cat[hsl].reshape(MH, 128, 128).transpose(2, 0, 1)
            .reshape(128, MH * 128)).astype(BF)
        # a2p[m]: lhsT = A2cat[:, hh-local m-tile].T -> [128(H-rows), 128(er)]
        a2ph = np.ascontiguousarray(
            A2cat[:, hsl].T.reshape(MH, 128, 128).transpose(1, 0, 2)
            .reshape(128, MH * 128)).astype(BF)
        bigc_np = np.concatenate([
            xcp.astype(BF),
            gt,
            a1p,
            np.ascontiguousarray(m1xf[tsl].T).astype(BF),
            np.ascontiguousarray(m2xf[tsl].T).astype(BF),
            b1ph,
            a2ph,
            b2p,
        ], axis=1)
        sm8 = np.concatenate([
            np.ascontiguousarray(dm8f[:, tsl]),
            np.ones((8, 128), np.float32),
        ], axis=1)
        smc = np.concatenate([
            np.ascontiguousarray(b1c_full[:, msl]),
            b2c if hh == 0 else b2c_zero,
        ], axis=1)
        w1c = np.ascontiguousarray(
            w1p_full[msl].reshape(MH // W1CH, W1CH, 128, KD * 128)
            .transpose(0, 2, 1, 3).reshape(MH // W1CH, 128, W1CH * KD * 128))
        # w2 chunk c covers m2 in [c*W2CH, (c+1)*W2CH), flattened (m2, k2)-major
        w2h = w2p_full[:, :, hh * MH * 128:(hh + 1) * MH * 128]     # [MD,128,MH*128]
        w2c = np.ascontiguousarray(
            w2h.reshape(MD // W2CH, W2CH, 128, MH * 128)
            .transpose(0, 2, 1, 3).reshape(MD // W2CH, 128, W2CH * MH * 128))
        in_maps.append({
            "bigc": bigc_np,
            "sm8": sm8,
            "smc": smc,
            "w1p": w1c,
            "w2p": w2c,
        })
    return in_maps, np.arange(NT), 0


_NC_CACHE = {}


def get_nc(slots=0):
    if slots not in _NC_CACHE:
        _NC_CACHE[slots] = _build_bass(slots)
    return _NC_CACHE[slots]


def _unpack_outputs(results, perm):
    cols = []
    for tq in range(TQ):
        o = None
        for hh in range(HH):
            c = tq * HH + hh
            p = np.asarray(results[c]["outt"], np.float32)
            p = p.reshape(128, MD, T).transpose(1, 0, 2).reshape(D, T)
            o = p if o is None else o + p
        cols.append(o)
    outT = np.concatenate(cols, axis=1)                  # [D, NT]
    out = np.empty((NT, D), np.float32)
    out[perm] = outT.T
    return out.reshape(2, NT // 2, D)


def kernel(**inputs):
    in_maps, perm, slots = _pack_inputs(**inputs)
    nc = get_nc(slots)
    res = run_bass_kernel_spmd(nc, in_maps, core_ids=list(range(NCORES)))
    return _unpack_outputs(res.results, perm)
